# revision 1
# baseline (speedup 1.0000x reference)
"""AcausalCrosscoder (topk) Trainium2 kernel — 8-core data-parallel over batch.

Math (per batch row):
    pre  = X @ W_enc + b_enc          (X: [B, D=MLD=8192], W_enc: [D, H=16384])
    h    = topk_mask(pre, k=64)       (keep top-64 per row, zero elsewhere)
    out  = h @ W_dec + b_dec          (W_dec: [H, D])

Precision scheme (matches fp32 top-k selection; see precision_sim.py):
    X = Xr + dX, W = Wr + dW with Xr/Wr rounded to fp32r (1s8e11m).
    pre = Xr@Wr (fp32r matmul, exact products, fp32 PSUM accumulation)
        + bf16(dX)@bf16(W) + bf16(X)@bf16(dW)   (bf16 corrections)
    dropped terms are O(2^-21) relative -> selection matches fp32.
    Decode runs in bf16 (output error ~2.5e-3 of absmax scale).

Each core handles B/8 = 512 rows; weights are replicated. No collectives.
"""

import numpy as np

import ml_dtypes

import concourse.bass as bass
import concourse.mybir as mybir
import concourse.tile as tile
from concourse import bacc
from concourse.bass import ds
from concourse.bass_utils import run_bass_kernel_spmd
from concourse.masks import make_identity

N_CORES = 8
_B, _M, _L, _DM = 4096, 2, 4, 1024
_D = _M * _L * _DM  # 8192
_H = 16384
_TOPK = 64


def round_fp32r(x: np.ndarray) -> np.ndarray:
    """Round fp32 -> fp32r (1s, 8e, 11m stored in top 20 bits), RNE."""
    x = np.ascontiguousarray(x, np.float32)
    bits = x.view(np.uint32).astype(np.uint64)
    lsb = (bits >> np.uint64(12)) & np.uint64(1)
    rounded = (bits + np.uint64(0x7FF) + lsb) & np.uint64(0xFFFFF000)
    return rounded.astype(np.uint32).view(np.float32).reshape(x.shape)


def _bf16(x: np.ndarray) -> np.ndarray:
    return np.asarray(x, np.float32).astype(ml_dtypes.bfloat16)


def build_nc(BC=_B // N_CORES, D=_D, H=_H, topk=_TOPK, decode_dt="bfloat16"):
    """Build the per-core Bass program (SPMD; all cores run the same NEFF)."""
    f32 = mybir.dt.float32
    f32r = mybir.dt.float32r
    bf = mybir.dt.bfloat16
    ddt = getattr(mybir.dt, decode_dt)

    KT = D // 128    # encode contraction chunks
    KH = H // 128    # decode contraction chunks
    MB = BC // 128   # batch blocks per core
    HG = H // 1024   # encode h-groups (2 x 512 tiles each)
    NG = D // 1024   # decode n-groups (2 x 512 tiles each)
    assert topk == 64 and MB >= 1 and HG >= 1 and NG >= 1

    nc = bacc.Bacc("TRN2", target_bir_lowering=False)

    xtr_p = nc.declare_dram_parameter("xtr", [D, BC], f32r, isOutput=False)
    xtb_p = nc.declare_dram_parameter("xtb", [D, BC], bf, isOutput=False)
    dxtb_p = nc.declare_dram_parameter("dxtb", [D, BC], bf, isOutput=False)
    wr_p = nc.declare_dram_parameter("wr", [D, H], f32r, isOutput=False)
    wb_p = nc.declare_dram_parameter("wb", [D, H], bf, isOutput=False)
    dwb_p = nc.declare_dram_parameter("dwb", [D, H], bf, isOutput=False)
    wd_p = nc.declare_dram_parameter("wd", [H, D], ddt, isOutput=False)
    benc_p = nc.declare_dram_parameter("benc", [H], f32, isOutput=False)
    bdec_p = nc.declare_dram_parameter("bdec", [D], f32, isOutput=False)
    out_p = nc.declare_dram_parameter("out", [BC, D], f32, isOutput=True)

    pre_dram = nc.dram_tensor("pre_scratch", [MB, 128, H], f32)
    ht_dram = nc.dram_tensor("ht_scratch", [KH, 128, BC], ddt)

    with tile.TileContext(nc) as tc:
        # ---------------- phase 1: encode ----------------
        with (
            tc.tile_pool(name="xres", bufs=1) as xres,
            tc.tile_pool(name="wstream", bufs=5) as wpool,
            tc.tile_pool(name="xstream", bufs=4) as xpool,
            tc.tile_pool(name="epsum", bufs=8, space="PSUM") as pse,
            tc.tile_pool(name="evac", bufs=4) as evac,
            tc.tile_pool(name="ebias", bufs=2) as ebias,
        ):
            xtb = xres.tile([128, KT, BC], bf, name="xtb_res")
            nc.sync.dma_start(xtb[:], xtb_p.rearrange("(kt p) b -> p kt b", p=128))
            dxtb = xres.tile([128, KT, BC], bf, name="dxtb_res")
            nc.sync.dma_start(dxtb[:], dxtb_p.rearrange("(kt p) b -> p kt b", p=128))

            for hg in range(HG):
                pss = [
                    [
                        pse.tile([128, 512], f32, name=f"eps_{m}_{j}", tag="eps")
                        for j in range(2)
                    ]
                    for m in range(MB)
                ]
                benc_t = ebias.tile([128, 2, 512], f32, tag="benc", name="benc_t")
                nc.scalar.dma_start(
                    benc_t[:],
                    benc_p[ds(hg * 1024, 1024)]
                    .rearrange("(j n) -> j n", n=512)
                    .partition_broadcast(128),
                )
                for kt in range(KT):
                    wr_t = wpool.tile([128, 2, 512], f32r, tag="wr", name="wr_t")
                    nc.sync.dma_start(
                        wr_t[:],
                        wr_p[ds(kt * 128, 128), ds(hg * 1024, 1024)].rearrange(
                            "p (j n) -> p j n", n=512
                        ),
                    )
                    wb_t = wpool.tile([128, 2, 512], bf, tag="wb", name="wb_t")
                    nc.gpsimd.dma_start(
                        wb_t[:],
                        wb_p[ds(kt * 128, 128), ds(hg * 1024, 1024)].rearrange(
                            "p (j n) -> p j n", n=512
                        ),
                    )
                    dwb_t = wpool.tile([128, 2, 512], bf, tag="dwb", name="dwb_t")
                    nc.scalar.dma_start(
                        dwb_t[:],
                        dwb_p[ds(kt * 128, 128), ds(hg * 1024, 1024)].rearrange(
                            "p (j n) -> p j n", n=512
                        ),
                    )
                    xtr_t = xpool.tile([128, BC], f32r, tag="xtr", name="xtr_t")
                    nc.scalar.dma_start(xtr_t[:], xtr_p[ds(kt * 128, 128), :])

                    first = kt == 0
                    last = kt == KT - 1
                    for m in range(MB):
                        for j in range(2):
                            nc.tensor.matmul(
                                pss[m][j][:],
                                xtr_t[:, ds(m * 128, 128)],
                                wr_t[:, j],
                                start=first,
                                stop=False,
                            )
                    for m in range(MB):
                        for j in range(2):
                            nc.tensor.matmul(
                                pss[m][j][:],
                                dxtb[:, kt, ds(m * 128, 128)],
                                wb_t[:, j],
                                start=False,
                                stop=False,
                            )
                    for m in range(MB):
                        for j in range(2):
                            nc.tensor.matmul(
                                pss[m][j][:],
                                xtb[:, kt, ds(m * 128, 128)],
                                dwb_t[:, j],
                                start=False,
                                stop=last,
                            )
                for m in range(MB):
                    for j in range(2):
                        st = evac.tile([128, 512], f32, tag="est", name="est")
                        nc.vector.tensor_add(st[:], pss[m][j][:], benc_t[:, j])
                        nc.scalar.dma_start(
                            pre_dram[m, :, ds(hg * 1024 + j * 512, 512)], st[:]
                        )

        # ---------------- phase 2: top-k mask + transpose ----------------
        with (
            tc.tile_pool(name="tk", bufs=2) as tk,
            tc.tile_pool(name="tk1", bufs=1) as tk1,
            tc.tile_pool(name="tk8", bufs=2) as tk8,
            tc.tile_pool(name="tpsum", bufs=4, space="PSUM") as tpsum,
            tc.tile_pool(name="hstage", bufs=6) as hstage,
        ):
            ident = tk1.tile([128, 128], f32, name="ident")
            make_identity(nc, ident[:])
            for m in range(MB):
                P = tk.tile([128, H], f32, tag="P", name="P")
                for pc in range(4):
                    nc.sync.dma_start(
                        P[:, ds(pc * (H // 4), H // 4)],
                        pre_dram[m, :, ds(pc * (H // 4), H // 4)],
                    )
                C = tk1.tile([128, H], f32, tag="C", name="C")
                # 8 rounds of (top-8 extract, zero them out) -> top-64 zeroed in C
                for r in range(8):
                    src = P if r == 0 else C
                    m8 = tk8.tile([128, 8], f32, tag="m8", name="m8")
                    nc.vector.max(m8[:], src[:])
                    nc.vector.match_replace(C[:], m8[:], src[:], 0.0)
                # hidden = P - C  (exactly the top-64 values, zero elsewhere;
                # valid because the 64th largest is always > 0 for this data)
                nc.vector.tensor_sub(C[:], P[:], C[:])
                for kh in range(KH):
                    pst = tpsum.tile([128, 128], f32, tag="tps", name="tps")
                    nc.tensor.transpose(pst[:], C[:, ds(kh * 128, 128)], ident[:])
                    hs = hstage.tile([128, 128], ddt, tag="hs", name="hs")
                    nc.scalar.copy(hs[:], pst[:])
                    nc.scalar.dma_start(ht_dram[kh, :, ds(m * 128, 128)], hs[:])

        # ---------------- phase 3: decode ----------------
        # m-pair-major: W_dec is streamed twice, but decode of (m0,m1) only
        # depends on their top-k, so it overlaps top-k of (m2,m3).
        with (
            tc.tile_pool(name="wdp", bufs=6) as wdp,
            tc.tile_pool(name="htp", bufs=8) as htp,
            tc.tile_pool(name="dpsum", bufs=8, space="PSUM") as dps,
            tc.tile_pool(name="devac", bufs=8) as devac,
            tc.tile_pool(name="dbias", bufs=2) as dbias,
        ):
            NT = D // 512
            for mp in range(max(MB // 2, 1)):
                ms = [mp * 2, mp * 2 + 1] if MB > 1 else [0]
                for ng in range(NT // 4):
                    pss = {
                        (m, j): dps.tile(
                            [128, 512], f32, name=f"dps_{m}_{j}", tag="dps"
                        )
                        for m in ms
                        for j in range(4)
                    }
                    bdec_t = dbias.tile([128, 4, 512], f32, tag="bdec", name="bdec_t")
                    nc.scalar.dma_start(
                        bdec_t[:],
                        bdec_p[ds(ng * 2048, 2048)]
                        .rearrange("(j n) -> j n", n=512)
                        .partition_broadcast(128),
                    )
                    for khb in range(KH // 4):
                        hts = {}
                        for m in ms:
                            ht_t = htp.tile(
                                [128, 4, 128], ddt, tag=f"ht{m % 2}", name=f"ht_t{m % 2}"
                            )
                            nc.gpsimd.dma_start(
                                ht_t[:],
                                ht_dram[
                                    ds(khb * 4, 4), :, ds(m * 128, 128)
                                ].rearrange("k p b -> p k b"),
                            )
                            hts[m] = ht_t
                        for k4 in range(4):
                            kh = khb * 4 + k4
                            wd_t = wdp.tile([128, 4, 512], ddt, tag="wd", name="wd_t")
                            nc.sync.dma_start(
                                wd_t[:],
                                wd_p[ds(kh * 128, 128), ds(ng * 2048, 2048)].rearrange(
                                    "p (j n) -> p j n", n=512
                                ),
                            )
                            first = kh == 0
                            last = kh == KH - 1
                            for m in ms:
                                for j in range(4):
                                    nc.tensor.matmul(
                                        pss[(m, j)][:],
                                        hts[m][:, k4],
                                        wd_t[:, j],
                                        start=first,
                                        stop=last,
                                    )
                    for m in ms:
                        for j in range(4):
                            st = devac.tile([128, 512], f32, tag="dst", name="dst")
                            nc.vector.tensor_add(st[:], pss[(m, j)][:], bdec_t[:, j])
                            nc.scalar.dma_start(
                                out_p[
                                    ds(m * 128, 128), ds(ng * 2048 + j * 512, 512)
                                ],
                                st[:],
                            )

    nc.compile()
    return nc


def prepare_inputs(X, W_enc, W_dec, b_enc, b_dec, n_cores=N_CORES):
    """Host-side dtype splits + per-core sharding. X: [B, D]."""
    B, D = X.shape
    H = W_enc.shape[1]
    BC = B // n_cores

    Wr = round_fp32r(W_enc)
    Wb = _bf16(W_enc)
    dWb = _bf16(W_enc - Wr)
    Wdb = _bf16(W_dec)
    benc = np.ascontiguousarray(b_enc, np.float32)
    bdec = np.ascontiguousarray(b_dec, np.float32).reshape(D)

    in_maps = []
    for c in range(n_cores):
        XT = np.ascontiguousarray(X[c * BC : (c + 1) * BC].T)  # [D, BC]
        XTr = round_fp32r(XT)
        in_maps.append(
            {
                "xtr": XTr,
                "xtb": _bf16(XT),
                "dxtb": _bf16(XT - XTr),
                "wr": Wr,
                "wb": Wb,
                "dwb": dWb,
                "wd": Wdb,
                "benc": benc,
                "bdec": bdec,
            }
        )
    return in_maps


_NC_CACHE = {}


def _get_nc(**kw):
    key = tuple(sorted(kw.items()))
    if key not in _NC_CACHE:
        _NC_CACHE[key] = build_nc(**kw)
    return _NC_CACHE[key]


def kernel(activation_BMLD, W_enc_MLDH, W_dec_HMLD, b_enc_H, b_dec_MLD, k, **run_kw):
    assert int(k) == _TOPK
    B = activation_BMLD.shape[0]
    X = np.ascontiguousarray(activation_BMLD, np.float32).reshape(B, _D)
    W_enc = np.ascontiguousarray(W_enc_MLDH, np.float32).reshape(_D, _H)
    W_dec = np.ascontiguousarray(W_dec_HMLD, np.float32).reshape(_H, _D)

    nc = _get_nc(BC=B // N_CORES)
    in_maps = prepare_inputs(X, W_enc, W_dec, b_enc_H, b_dec_MLD)
    res = run_bass_kernel_spmd(nc, in_maps, core_ids=list(range(N_CORES)), **run_kw)
    out = np.concatenate([res.results[c]["out"] for c in range(N_CORES)], axis=0)
    if run_kw.get("trace"):
        kernel.last_result = res
    return out.reshape(B, _M, _L, _DM).astype(np.float32)



# revision 4
# speedup vs baseline: 1.6175x; 1.6175x over previous
"""AcausalCrosscoder (topk) Trainium2 kernel — 8-core data-parallel over batch.

Math (per batch row):
    pre  = X @ W_enc + b_enc          (X: [B, D=MLD=8192], W_enc: [D, H=16384])
    h    = topk_mask(pre, k=64)       (keep top-64 per row, zero elsewhere)
    out  = h @ W_dec + b_dec          (W_dec: [H, D])

Precision scheme (fp32-exact top-k selection):
    X = Xr + dX, W = Wr + dW with Xr/Wr rounded to fp32r (1s8e11m).
    pre = Xr@Wr (fp32r matmul, exact products, fp32 PSUM accumulation)
        + fp8(dX*2^13)@fp8(W*2^9) + fp8(X)@fp8(dW*2^22)   (e4m3 DoubleRow
          corrections at 2x matmul rate, both at a common 2^22 product scale)
    The whole encode runs at a 2^22 scale (Xr*2^11 @ Wr*2^11) so main and
    corrections share one PSUM accumulation group; the scale is folded into
    W_dec (*2^-22) on the host so decode output is unscaled.
    Residual selection noise ~1.5e-6 vs a ~1.3e-3 median top-64 boundary gap.

Top-k: during encode evacuation each 512-wide pre chunk gets a top-16
candidate extraction (max8 + match_replace + max8, hidden under the PE);
after encode an 8-round merge of the [128, 512] candidate array yields the
64th-largest value t per row, and hidden = (pre >= t) * pre is applied with
one fused scalar_tensor_tensor op per tile. Decode streams W_dec once with
the transposed hidden resident in SBUF.

Each core handles B/8 = 512 rows; weights are replicated. No collectives.
"""

import numpy as np

import ml_dtypes

import concourse.bass as bass
import concourse.mybir as mybir
import concourse.tile as tile
from concourse import bacc
from concourse.bass import ds
from concourse.bass_utils import run_bass_kernel_spmd
from concourse.masks import make_identity

N_CORES = 8
_B, _M, _L, _DM = 4096, 2, 4, 1024
_D = _M * _L * _DM  # 8192
_H = 16384
_TOPK = 64


def round_fp32r(x: np.ndarray) -> np.ndarray:
    """Round fp32 -> fp32r (1s, 8e, 11m stored in top 20 bits), RNE."""
    x = np.ascontiguousarray(x, np.float32)
    bits = x.view(np.uint32).astype(np.uint64)
    lsb = (bits >> np.uint64(12)) & np.uint64(1)
    rounded = (bits + np.uint64(0x7FF) + lsb) & np.uint64(0xFFFFF000)
    return rounded.astype(np.uint32).view(np.float32).reshape(x.shape)


def _bf16(x: np.ndarray) -> np.ndarray:
    return np.asarray(x, np.float32).astype(ml_dtypes.bfloat16)


def _e4(x: np.ndarray) -> np.ndarray:
    return np.asarray(x, np.float32).astype(ml_dtypes.float8_e4m3)


def build_nc(BC=_B // N_CORES, D=_D, H=_H, topk=_TOPK, decode_dt="bfloat16"):
    """Build the per-core Bass program (SPMD; all cores run the same NEFF)."""
    f32 = mybir.dt.float32
    f32r = mybir.dt.float32r
    e4 = mybir.dt.float8e4
    bf = mybir.dt.bfloat16
    ddt = getattr(mybir.dt, decode_dt)
    DR = mybir.MatmulPerfMode.DoubleRow
    is_ge = mybir.AluOpType.is_ge
    mult = mybir.AluOpType.mult

    KT = D // 128    # 64 encode contraction chunks
    KT2 = KT // 2    # 32 fp8 pair chunks
    KH = H // 128    # 128 decode contraction chunks
    MB = BC // 128   # 4 batch blocks per core
    HG = H // 1024   # 16 encode h-groups (2 x 512 tiles each)
    NCH = H // 512   # 32 candidate chunks per row
    assert topk == 64 and MB == 4

    nc = bacc.Bacc("TRN2", target_bir_lowering=False)

    xtr_p = nc.declare_dram_parameter("xtr", [D, BC], f32r, isOutput=False)
    x8_p = nc.declare_dram_parameter("x8", [D, BC], e4, isOutput=False)
    dx8_p = nc.declare_dram_parameter("dx8", [D, BC], e4, isOutput=False)
    wr_p = nc.declare_dram_parameter("wr", [D, H], f32r, isOutput=False)
    w8_p = nc.declare_dram_parameter("w8", [D, H], e4, isOutput=False)
    dw8_p = nc.declare_dram_parameter("dw8", [D, H], e4, isOutput=False)
    wd_p = nc.declare_dram_parameter("wd", [H, D], ddt, isOutput=False)
    benc_p = nc.declare_dram_parameter("benc", [H], f32, isOutput=False)
    bdec_p = nc.declare_dram_parameter("bdec", [D], f32, isOutput=False)
    out_p = nc.declare_dram_parameter("out", [BC, D], f32, isOutput=True)

    pre_dram = nc.dram_tensor("pre_scratch", [MB, 128, H], f32)

    with tile.TileContext(nc) as tc:
        with tc.tile_pool(name="persist", bufs=1) as pers:
            ident = pers.tile([128, 128], f32, name="ident")
            make_identity(nc, ident[:])
            cands = [
                pers.tile([128, NCH, 16], f32, name=f"cand{m}") for m in range(MB)
            ]
            m8s = [pers.tile([128, 8], f32, name=f"m8_{m}") for m in range(MB)]

            # ---------------- phase 1: encode ----------------
            with (
                tc.tile_pool(name="xres", bufs=1) as xres,
                tc.tile_pool(name="wstream", bufs=5) as wpool,
                tc.tile_pool(name="w8stream", bufs=5) as w8pool,
                tc.tile_pool(name="xstream", bufs=4) as xpool,
                tc.tile_pool(name="epsum", bufs=8, space="PSUM") as pse,
                tc.tile_pool(name="evac", bufs=4) as evac,
                tc.tile_pool(name="cscr", bufs=2) as cscr,
                tc.tile_pool(name="ebias", bufs=2) as ebias,
            ):
                x8 = xres.tile([128, KT, BC], e4, name="x8_res")
                nc.sync.dma_start(x8[:], x8_p.rearrange("(kt p) b -> p kt b", p=128))
                dx8 = xres.tile([128, KT, BC], e4, name="dx8_res")
                nc.sync.dma_start(dx8[:], dx8_p.rearrange("(kt p) b -> p kt b", p=128))

                for hg in range(HG):
                    pss = [
                        [
                            pse.tile([128, 512], f32, name=f"eps_{m}_{j}", tag="eps")
                            for j in range(2)
                        ]
                        for m in range(MB)
                    ]
                    benc_t = ebias.tile([128, 2, 512], f32, tag="benc", name="benc_t")
                    nc.scalar.dma_start(
                        benc_t[:],
                        benc_p[ds(hg * 1024, 1024)]
                        .rearrange("(j n) -> j n", n=512)
                        .partition_broadcast(128),
                    )
                    for kt2 in range(KT2):
                        wr_t = wpool.tile([128, 2, 1024], f32r, tag="wr", name="wr_t")
                        nc.sync.dma_start(
                            wr_t[:],
                            wr_p[ds(kt2 * 256, 256), ds(hg * 1024, 1024)].rearrange(
                                "(k p) n -> p k n", p=128
                            ),
                        )
                        w8_t = w8pool.tile([128, 2, 1024], e4, tag="w8", name="w8_t")
                        nc.gpsimd.dma_start(
                            w8_t[:],
                            w8_p[ds(kt2 * 256, 256), ds(hg * 1024, 1024)].rearrange(
                                "(k p) n -> p k n", p=128
                            ),
                        )
                        dw8_t = w8pool.tile([128, 2, 1024], e4, tag="dw8", name="dw8_t")
                        nc.gpsimd.dma_start(
                            dw8_t[:],
                            dw8_p[ds(kt2 * 256, 256), ds(hg * 1024, 1024)].rearrange(
                                "(k p) n -> p k n", p=128
                            ),
                        )
                        xtr_t = xpool.tile([128, 2, BC], f32r, tag="xtr", name="xtr_t")
                        nc.scalar.dma_start(
                            xtr_t[:],
                            xtr_p[ds(kt2 * 256, 256), :].rearrange(
                                "(k p) b -> p k b", p=128
                            ),
                        )

                        first = kt2 == 0
                        last = kt2 == KT2 - 1
                        for kk in range(2):
                            for m in range(MB):
                                for j in range(2):
                                    nc.tensor.matmul(
                                        pss[m][j][:],
                                        xtr_t[:, kk, ds(m * 128, 128)],
                                        wr_t[:, kk, ds(j * 512, 512)],
                                        start=(first and kk == 0),
                                        stop=False,
                                    )
                        for m in range(MB):
                            for j in range(2):
                                nc.tensor.matmul(
                                    pss[m][j][:],
                                    dx8[:, ds(kt2 * 2, 2), ds(m * 128, 128)],
                                    w8_t[:, :, ds(j * 512, 512)],
                                    start=False,
                                    stop=False,
                                    perf_mode=DR,
                                )
                        for m in range(MB):
                            for j in range(2):
                                nc.tensor.matmul(
                                    pss[m][j][:],
                                    x8[:, ds(kt2 * 2, 2), ds(m * 128, 128)],
                                    dw8_t[:, :, ds(j * 512, 512)],
                                    start=False,
                                    stop=last,
                                    perf_mode=DR,
                                )
                    for m in range(MB):
                        for j in range(2):
                            st = evac.tile([128, 512], f32, tag="est", name="est")
                            nc.vector.tensor_add(st[:], pss[m][j][:], benc_t[:, j])
                            nc.scalar.dma_start(
                                pre_dram[m, :, ds(hg * 1024 + j * 512, 512)], st[:]
                            )
                            # top-16 candidate extraction for this 512 chunk
                            c = hg * 2 + j
                            cs = cscr.tile([128, 512], f32, tag="cs", name="cs")
                            nc.vector.max(cands[m][:, c, ds(0, 8)], st[:])
                            nc.vector.match_replace(
                                cs[:], cands[m][:, c, ds(0, 8)], st[:], 0.0
                            )
                            nc.vector.max(cands[m][:, c, ds(8, 8)], cs[:])

            # -------- phase 2+3 container: ht stays resident in SBUF --------
            with tc.tile_pool(name="htres", bufs=1) as htres:
                ht = htres.tile([128, KH, BC], ddt, name="ht_res")

                # ---- phase 2: merge candidates -> threshold -> hidden^T ----
                with (
                    tc.tile_pool(name="prel", bufs=3) as prel,
                    tc.tile_pool(name="hidp", bufs=3) as hidp,
                    tc.tile_pool(name="tpsum", bufs=8, space="PSUM") as tps,
                ):
                    E8 = 2048
                    NE = H // E8  # 8
                    for m in range(MB):
                        for r in range(8):
                            nc.vector.max(m8s[m][:], cands[m][:])
                            if r < 7:
                                nc.vector.match_replace(
                                    cands[m][:], m8s[m][:], cands[m][:], 0.0
                                )
                        t_ap = m8s[m][:, ds(7, 1)]
                        for e in range(NE):
                            pt = prel.tile([128, E8], f32, tag="pt", name="pt")
                            nc.sync.dma_start(pt[:], pre_dram[m, :, ds(e * E8, E8)])
                            hd = hidp.tile([128, E8], f32, tag="hd", name="hd")
                            nc.vector.scalar_tensor_tensor(
                                hd[:], pt[:], t_ap, pt[:], is_ge, mult
                            )
                            for kb in range(E8 // 128):
                                kh = e * (E8 // 128) + kb
                                pst = tps.tile([128, 128], f32, tag="tps", name="tps")
                                nc.tensor.transpose(
                                    pst[:], hd[:, ds(kb * 128, 128)], ident[:]
                                )
                                if kh % 2 == 0:
                                    nc.scalar.copy(
                                        ht[:, kh, ds(m * 128, 128)], pst[:]
                                    )
                                else:
                                    nc.vector.tensor_scalar_add(
                                        ht[:, kh, ds(m * 128, 128)], pst[:], 0.0
                                    )

                # ---------------- phase 3: decode ----------------
                with (
                    tc.tile_pool(name="wdp", bufs=8) as wdp,
                    tc.tile_pool(name="dpsum", bufs=8, space="PSUM") as dps,
                    tc.tile_pool(name="devac", bufs=6) as devac,
                    tc.tile_pool(name="dbias", bufs=2) as dbias,
                ):
                    NG = D // 1024  # 8
                    for ng in range(NG):
                        pss = [
                            [
                                dps.tile(
                                    [128, 512], f32, name=f"dps_{m}_{j}", tag="dps"
                                )
                                for j in range(2)
                            ]
                            for m in range(MB)
                        ]
                        bdec_t = dbias.tile(
                            [128, 2, 512], f32, tag="bdec", name="bdec_t"
                        )
                        nc.scalar.dma_start(
                            bdec_t[:],
                            bdec_p[ds(ng * 1024, 1024)]
                            .rearrange("(j n) -> j n", n=512)
                            .partition_broadcast(128),
                        )
                        for kh in range(KH):
                            wd_t = wdp.tile([128, 1024], ddt, tag="wd", name="wd_t")
                            nc.sync.dma_start(
                                wd_t[:, ds(0, 512)],
                                wd_p[ds(kh * 128, 128), ds(ng * 1024, 512)],
                            )
                            nc.gpsimd.dma_start(
                                wd_t[:, ds(512, 512)],
                                wd_p[ds(kh * 128, 128), ds(ng * 1024 + 512, 512)],
                            )
                            first = kh == 0
                            last = kh == KH - 1
                            for m in range(MB):
                                for j in range(2):
                                    nc.tensor.matmul(
                                        pss[m][j][:],
                                        ht[:, kh, ds(m * 128, 128)],
                                        wd_t[:, ds(j * 512, 512)],
                                        start=first,
                                        stop=last,
                                    )
                        for m in range(MB):
                            for j in range(2):
                                st = devac.tile([128, 512], f32, tag="dst", name="dst")
                                nc.vector.tensor_add(st[:], pss[m][j][:], bdec_t[:, j])
                                nc.scalar.dma_start(
                                    out_p[
                                        ds(m * 128, 128), ds(ng * 1024 + j * 512, 512)
                                    ],
                                    st[:],
                                )

    nc.compile()
    return nc


def prepare_inputs(X, W_enc, W_dec, b_enc, b_dec, n_cores=N_CORES):
    """Host-side dtype splits + per-core sharding. X: [B, D]."""
    B, D = X.shape
    BC = B // n_cores

    Wr = round_fp32r(W_enc)
    wr = (Wr * np.float32(2048.0)).astype(np.float32)  # fp32r * 2^11
    w8 = _e4(W_enc * np.float32(2.0**9))
    dw8 = _e4((W_enc - Wr) * np.float32(2.0**22))
    wd = _bf16(np.asarray(W_dec, np.float32) * np.float32(2.0**-22))
    benc = (np.ascontiguousarray(b_enc, np.float32) * np.float32(2.0**22)).astype(
        np.float32
    )
    bdec = np.ascontiguousarray(b_dec, np.float32).reshape(D)

    in_maps = []
    for c in range(n_cores):
        XT = np.ascontiguousarray(X[c * BC : (c + 1) * BC].T)  # [D, BC]
        XTr = round_fp32r(XT)
        in_maps.append(
            {
                "xtr": (XTr * np.float32(2048.0)).astype(np.float32),
                "x8": _e4(XT),
                "dx8": _e4((XT - XTr) * np.float32(2.0**13)),
                "wr": wr,
                "w8": w8,
                "dw8": dw8,
                "wd": wd,
                "benc": benc,
                "bdec": bdec,
            }
        )
    return in_maps


_NC_CACHE = {}


def _get_nc(**kw):
    key = tuple(sorted(kw.items()))
    if key not in _NC_CACHE:
        _NC_CACHE[key] = build_nc(**kw)
    return _NC_CACHE[key]


def kernel(activation_BMLD, W_enc_MLDH, W_dec_HMLD, b_enc_H, b_dec_MLD, k, **run_kw):
    assert int(k) == _TOPK
    B = activation_BMLD.shape[0]
    X = np.ascontiguousarray(activation_BMLD, np.float32).reshape(B, _D)
    W_enc = np.ascontiguousarray(W_enc_MLDH, np.float32).reshape(_D, _H)
    W_dec = np.ascontiguousarray(W_dec_HMLD, np.float32).reshape(_H, _D)

    nc = _get_nc(BC=B // N_CORES)
    in_maps = prepare_inputs(X, W_enc, W_dec, b_enc_H, b_dec_MLD)
    res = run_bass_kernel_spmd(nc, in_maps, core_ids=list(range(N_CORES)), **run_kw)
    out = np.concatenate([res.results[c]["out"] for c in range(N_CORES)], axis=0)
    if run_kw.get("trace"):
        kernel.last_result = res
    return out.reshape(B, _M, _L, _DM).astype(np.float32)


# revision 12
# speedup vs baseline: 1.6566x; 1.0241x over previous
"""AcausalCrosscoder (topk) Trainium2 kernel — 8-core data-parallel over batch.

Math (per batch row):
    pre  = X @ W_enc + b_enc          (X: [B, D=MLD=8192], W_enc: [D, H=16384])
    h    = topk_mask(pre, k=64)       (keep top-64 per row, zero elsewhere)
    out  = h @ W_dec + b_dec          (W_dec: [H, D])

Precision scheme (fp32-exact top-k selection):
    X = Xr + dX, W = Wr + dW with Xr/Wr rounded to fp32r (1s8e11m).
    pre = Xr@Wr (fp32r matmul, exact products, fp32 PSUM accumulation)
        + fp8(dX*2^13)@fp8(W*2^9) + fp8(X)@fp8(dW*2^22)   (e4m3 DoubleRow
          corrections at 2x matmul rate, both at a common 2^22 product scale)
    The whole encode runs at a 2^22 scale (Xr*2^11 @ Wr*2^11) so main and
    corrections share one PSUM accumulation group; the scale is folded into
    W_dec (*2^-22) on the host so decode output is unscaled.
    Residual selection noise ~1.5e-6 vs a ~1.3e-3 median top-64 boundary gap.

Top-k: during encode evacuation each 512-wide pre chunk gets a top-16
candidate extraction (max8 + match_replace + max8, hidden under the PE);
after encode an 8-round merge of the [128, 512] candidate array yields the
64th-largest value t per row, and hidden = (pre >= t) * pre is applied with
one fused scalar_tensor_tensor op per tile. Decode streams W_dec once with
the transposed hidden resident in SBUF.

Each core handles B/8 = 512 rows; weights are replicated. No collectives.
"""

import numpy as np

import ml_dtypes

import concourse.bass as bass
import concourse.mybir as mybir
import concourse.tile as tile
from concourse import bacc
from concourse.bass import ds
from concourse.bass_utils import run_bass_kernel_spmd
from concourse.masks import make_identity

N_CORES = 8
_B, _M, _L, _DM = 4096, 2, 4, 1024
_D = _M * _L * _DM  # 8192
_H = 16384
_TOPK = 64


def round_fp32r(x: np.ndarray) -> np.ndarray:
    """Round fp32 -> fp32r (1s, 8e, 11m stored in top 20 bits), RNE."""
    x = np.ascontiguousarray(x, np.float32)
    bits = x.view(np.uint32).astype(np.uint64)
    lsb = (bits >> np.uint64(12)) & np.uint64(1)
    rounded = (bits + np.uint64(0x7FF) + lsb) & np.uint64(0xFFFFF000)
    return rounded.astype(np.uint32).view(np.float32).reshape(x.shape)


def _bf16(x: np.ndarray) -> np.ndarray:
    return np.asarray(x, np.float32).astype(ml_dtypes.bfloat16)


def _e4(x: np.ndarray) -> np.ndarray:
    return np.asarray(x, np.float32).astype(ml_dtypes.float8_e4m3)


def build_nc(BC=_B // N_CORES, D=_D, H=_H, topk=_TOPK, decode_dt="bfloat16"):
    """Build the per-core Bass program (SPMD; all cores run the same NEFF)."""
    f32 = mybir.dt.float32
    f32r = mybir.dt.float32r
    e4 = mybir.dt.float8e4
    bf = mybir.dt.bfloat16
    ddt = getattr(mybir.dt, decode_dt)
    DR = mybir.MatmulPerfMode.DoubleRow
    is_ge = mybir.AluOpType.is_ge
    mult = mybir.AluOpType.mult

    KT = D // 128    # 64 encode contraction chunks
    KT2 = KT // 2    # 32 fp8 pair chunks
    KH = H // 128    # 128 decode contraction chunks
    MB = BC // 128   # 4 batch blocks per core
    HG = H // 1024   # 16 encode h-groups (2 x 512 tiles each)
    NCH = H // 512   # 32 candidate chunks per row
    assert topk == 64 and MB == 4

    nc = bacc.Bacc("TRN2", target_bir_lowering=False)

    xtr_p = nc.declare_dram_parameter("xtr", [D, BC], f32r, isOutput=False)
    x8_p = nc.declare_dram_parameter("x8", [D, BC], e4, isOutput=False)
    dx8_p = nc.declare_dram_parameter("dx8", [D, BC], e4, isOutput=False)
    wr_p = nc.declare_dram_parameter("wr", [D, H], f32r, isOutput=False)
    w8_p = nc.declare_dram_parameter("w8", [D, H], e4, isOutput=False)
    dw8_p = nc.declare_dram_parameter("dw8", [D, H], e4, isOutput=False)
    wd_p = nc.declare_dram_parameter("wd", [H, D], ddt, isOutput=False)
    benc_p = nc.declare_dram_parameter("benc", [H], f32, isOutput=False)
    bdec_p = nc.declare_dram_parameter("bdec", [D], f32, isOutput=False)
    out_p = nc.declare_dram_parameter("out", [BC, D], f32, isOutput=True)

    pre_dram = nc.dram_tensor("pre_scratch", [MB, 128, H], f32)

    with tile.TileContext(nc) as tc:
        with tc.tile_pool(name="persist", bufs=1) as pers:
            ident = pers.tile([128, 128], f32, name="ident")
            make_identity(nc, ident[:])
            cands = [
                pers.tile([128, NCH, 16], f32, name=f"cand{m}") for m in range(MB)
            ]
            m8s = [pers.tile([128, 8], f32, name=f"m8_{m}") for m in range(MB)]

            # ---------------- phase 1: encode ----------------
            with (
                tc.tile_pool(name="xres", bufs=1) as xres,
                tc.tile_pool(name="wstream", bufs=5) as wpool,
                tc.tile_pool(name="w8stream", bufs=5) as w8pool,
                tc.tile_pool(name="xstream", bufs=4) as xpool,
                tc.tile_pool(name="epsum", bufs=8, space="PSUM") as pse,
                tc.tile_pool(name="evac", bufs=10) as evac,
                tc.tile_pool(name="cscr", bufs=2) as cscr,
                tc.tile_pool(name="ebias", bufs=2) as ebias,
            ):
                # resident fp8 X tiles, loaded in chunks so the first fp8
                # matmuls don't wait on the full 16MB transfer
                x8 = xres.tile([128, KT, BC], e4, name="x8_res")
                dx8 = xres.tile([128, KT, BC], e4, name="dx8_res")
                XCH = 16
                for ch in range(XCH):
                    kl = KT // XCH
                    nc.vector.dma_start(
                        dx8[:, ds(ch * kl, kl), :],
                        dx8_p[ds(ch * kl * 128, kl * 128), :].rearrange(
                            "(kt p) b -> p kt b", p=128
                        ),
                    )
                    nc.vector.dma_start(
                        x8[:, ds(ch * kl, kl), :],
                        x8_p[ds(ch * kl * 128, kl * 128), :].rearrange(
                            "(kt p) b -> p kt b", p=128
                        ),
                    )

                for hg in range(HG):
                    pss = [
                        [
                            pse.tile([128, 512], f32, name=f"eps_{m}_{j}", tag="eps")
                            for j in range(2)
                        ]
                        for m in range(MB)
                    ]
                    benc_t = ebias.tile([128, 2, 512], f32, tag="benc", name="benc_t")
                    nc.scalar.dma_start(
                        benc_t[:],
                        benc_p[ds(hg * 1024, 1024)]
                        .rearrange("(j n) -> j n", n=512)
                        .partition_broadcast(128),
                    )
                    for kt2 in range(KT2):
                        wr_t = wpool.tile([128, 2, 1024], f32r, tag="wr", name="wr_t")
                        nc.sync.dma_start(
                            wr_t[:],
                            wr_p[ds(kt2 * 256, 256), ds(hg * 1024, 1024)].rearrange(
                                "(k p) n -> p k n", p=128
                            ),
                        )
                        w8_t = w8pool.tile([128, 2, 1024], e4, tag="w8", name="w8_t")
                        nc.gpsimd.dma_start(
                            w8_t[:],
                            w8_p[ds(kt2 * 256, 256), ds(hg * 1024, 1024)].rearrange(
                                "(k p) n -> p k n", p=128
                            ),
                        )
                        dw8_t = w8pool.tile([128, 2, 1024], e4, tag="dw8", name="dw8_t")
                        nc.gpsimd.dma_start(
                            dw8_t[:],
                            dw8_p[ds(kt2 * 256, 256), ds(hg * 1024, 1024)].rearrange(
                                "(k p) n -> p k n", p=128
                            ),
                        )
                        xtr_t = xpool.tile([128, 2, BC], f32r, tag="xtr", name="xtr_t")
                        nc.scalar.dma_start(
                            xtr_t[:],
                            xtr_p[ds(kt2 * 256, 256), :].rearrange(
                                "(k p) b -> p k b", p=128
                            ),
                        )

                        first = kt2 == 0
                        last = kt2 == KT2 - 1
                        for kk in range(2):
                            for m in range(MB):
                                for j in range(2):
                                    nc.tensor.matmul(
                                        pss[m][j][:],
                                        xtr_t[:, kk, ds(m * 128, 128)],
                                        wr_t[:, kk, ds(j * 512, 512)],
                                        start=(first and kk == 0),
                                        stop=False,
                                    )
                        for m in range(MB):
                            for j in range(2):
                                nc.tensor.matmul(
                                    pss[m][j][:],
                                    dx8[:, ds(kt2 * 2, 2), ds(m * 128, 128)],
                                    w8_t[:, :, ds(j * 512, 512)],
                                    start=False,
                                    stop=False,
                                    perf_mode=DR,
                                )
                        for m in range(MB):
                            for j in range(2):
                                nc.tensor.matmul(
                                    pss[m][j][:],
                                    x8[:, ds(kt2 * 2, 2), ds(m * 128, 128)],
                                    dw8_t[:, :, ds(j * 512, 512)],
                                    start=False,
                                    stop=last,
                                    perf_mode=DR,
                                )
                    # free all 8 PSUM banks first (bias adds), then do the
                    # candidate extraction on the SBUF staging tiles so the
                    # next h-group's matmuls aren't gated on DVE extract work
                    sts = {}
                    for m in range(MB):
                        for j in range(2):
                            st = evac.tile([128, 512], f32, tag="est", name="est")
                            if (m * 2 + j) % 2 == 0:
                                nc.vector.tensor_add(st[:], pss[m][j][:], benc_t[:, j])
                            else:
                                nc.scalar.activation(
                                    st[:],
                                    pss[m][j][:],
                                    mybir.ActivationFunctionType.Identity,
                                    bias=0.0,
                                    scale=1.0,
                                )
                                nc.vector.tensor_add(st[:], st[:], benc_t[:, j])
                            nc.scalar.dma_start(
                                pre_dram[m, :, ds(hg * 1024 + j * 512, 512)], st[:]
                            )
                            sts[(m, j)] = st
                    for m in range(MB):
                        for j in range(2):
                            # top-16 candidate extraction for this 512 chunk
                            st = sts[(m, j)]
                            c = hg * 2 + j
                            cs = cscr.tile([128, 512], f32, tag="cs", name="cs")
                            nc.vector.max(cands[m][:, c, ds(0, 8)], st[:])
                            nc.vector.match_replace(
                                cs[:], cands[m][:, c, ds(0, 8)], st[:], 0.0
                            )
                            nc.vector.max(cands[m][:, c, ds(8, 8)], cs[:])

            # -------- phase 2+3 container: ht stays resident in SBUF --------
            with tc.tile_pool(name="htres", bufs=1) as htres:
                ht = htres.tile([128, KH, BC], ddt, name="ht_res")

                # ---- phase 2: merge candidates -> threshold -> hidden^T ----
                with (
                    tc.tile_pool(name="prel", bufs=3) as prel,
                    tc.tile_pool(name="hidp", bufs=3) as hidp,
                    tc.tile_pool(name="tpsum", bufs=8, space="PSUM") as tps,
                ):
                    E8 = 2048
                    NE = H // E8  # 8
                    # 8-round merge of each m's candidate array -> 64th value
                    for m in range(MB):
                        for r in range(8):
                            nc.vector.max(m8s[m][:], cands[m][:])
                            if r < 7:
                                nc.vector.match_replace(
                                    cands[m][:], m8s[m][:], cands[m][:], 0.0
                                )
                    # e-outer so low-kh ht tiles complete first and decode's
                    # first contraction chunks can start against them
                    for e in range(NE):
                        for m in range(MB):
                            t_ap = m8s[m][:, ds(7, 1)]
                            pt = prel.tile([128, E8], f32, tag="pt", name="pt")
                            nc.sync.dma_start(pt[:], pre_dram[m, :, ds(e * E8, E8)])
                            hd = hidp.tile([128, E8], f32, tag="hd", name="hd")
                            nc.vector.scalar_tensor_tensor(
                                hd[:], pt[:], t_ap, pt[:], is_ge, mult
                            )
                            for kb in range(E8 // 128):
                                kh = e * (E8 // 128) + kb
                                pst = tps.tile([128, 128], f32, tag="tps", name="tps")
                                nc.tensor.transpose(
                                    pst[:], hd[:, ds(kb * 128, 128)], ident[:]
                                )
                                if kh % 2 == 0:
                                    nc.scalar.copy(
                                        ht[:, kh, ds(m * 128, 128)], pst[:]
                                    )
                                else:
                                    nc.vector.tensor_scalar_add(
                                        ht[:, kh, ds(m * 128, 128)], pst[:], 0.0
                                    )

                # ---------------- phase 3: decode ----------------
                with (
                    tc.tile_pool(name="wdp", bufs=8) as wdp,
                    tc.tile_pool(name="dpsum", bufs=8, space="PSUM") as dps,
                    tc.tile_pool(name="devac", bufs=6) as devac,
                    tc.tile_pool(name="dbias", bufs=2) as dbias,
                ):
                    NG = D // 1024  # 8
                    for ng in range(NG):
                        pss = [
                            [
                                dps.tile(
                                    [128, 512], f32, name=f"dps_{m}_{j}", tag="dps"
                                )
                                for j in range(2)
                            ]
                            for m in range(MB)
                        ]
                        bdec_t = dbias.tile(
                            [128, 2, 512], f32, tag="bdec", name="bdec_t"
                        )
                        nc.scalar.dma_start(
                            bdec_t[:],
                            bdec_p[ds(ng * 1024, 1024)]
                            .rearrange("(j n) -> j n", n=512)
                            .partition_broadcast(128),
                        )
                        for kh in range(KH):
                            wd_t = wdp.tile([128, 1024], ddt, tag="wd", name="wd_t")
                            nc.sync.dma_start(
                                wd_t[:, ds(0, 512)],
                                wd_p[ds(kh * 128, 128), ds(ng * 1024, 512)],
                            )
                            nc.gpsimd.dma_start(
                                wd_t[:, ds(512, 512)],
                                wd_p[ds(kh * 128, 128), ds(ng * 1024 + 512, 512)],
                            )
                            first = kh == 0
                            last = kh == KH - 1
                            for m in range(MB):
                                for j in range(2):
                                    nc.tensor.matmul(
                                        pss[m][j][:],
                                        ht[:, kh, ds(m * 128, 128)],
                                        wd_t[:, ds(j * 512, 512)],
                                        start=first,
                                        stop=last,
                                    )
                        for m in range(MB):
                            for j in range(2):
                                st = devac.tile([128, 512], f32, tag="dst", name="dst")
                                # split psum-freeing adds across DVE and ACT so
                                # the next ng group's matmuls unblock sooner
                                if (m * 2 + j) % 2 == 0:
                                    nc.vector.tensor_add(
                                        st[:], pss[m][j][:], bdec_t[:, j]
                                    )
                                else:
                                    nc.scalar.activation(
                                        st[:],
                                        pss[m][j][:],
                                        mybir.ActivationFunctionType.Identity,
                                        bias=0.0,
                                        scale=1.0,
                                    )
                                    nc.vector.tensor_add(st[:], st[:], bdec_t[:, j])
                                nc.scalar.dma_start(
                                    out_p[
                                        ds(m * 128, 128), ds(ng * 1024 + j * 512, 512)
                                    ],
                                    st[:],
                                )

    nc.compile()
    return nc


def prepare_inputs(X, W_enc, W_dec, b_enc, b_dec, n_cores=N_CORES):
    """Host-side dtype splits + per-core sharding. X: [B, D]."""
    B, D = X.shape
    BC = B // n_cores

    Wr = round_fp32r(W_enc)
    wr = (Wr * np.float32(2048.0)).astype(np.float32)  # fp32r * 2^11
    w8 = _e4(W_enc * np.float32(2.0**9))
    dw8 = _e4((W_enc - Wr) * np.float32(2.0**22))
    wd = _bf16(np.asarray(W_dec, np.float32) * np.float32(2.0**-22))
    benc = (np.ascontiguousarray(b_enc, np.float32) * np.float32(2.0**22)).astype(
        np.float32
    )
    bdec = np.ascontiguousarray(b_dec, np.float32).reshape(D)

    in_maps = []
    for c in range(n_cores):
        XT = np.ascontiguousarray(X[c * BC : (c + 1) * BC].T)  # [D, BC]
        XTr = round_fp32r(XT)
        in_maps.append(
            {
                "xtr": (XTr * np.float32(2048.0)).astype(np.float32),
                "x8": _e4(XT),
                "dx8": _e4((XT - XTr) * np.float32(2.0**13)),
                "wr": wr,
                "w8": w8,
                "dw8": dw8,
                "wd": wd,
                "benc": benc,
                "bdec": bdec,
            }
        )
    return in_maps


_NC_CACHE = {}


def _get_nc(**kw):
    key = tuple(sorted(kw.items()))
    if key not in _NC_CACHE:
        _NC_CACHE[key] = build_nc(**kw)
    return _NC_CACHE[key]


def kernel(activation_BMLD, W_enc_MLDH, W_dec_HMLD, b_enc_H, b_dec_MLD, k, **run_kw):
    assert int(k) == _TOPK
    B = activation_BMLD.shape[0]
    X = np.ascontiguousarray(activation_BMLD, np.float32).reshape(B, _D)
    W_enc = np.ascontiguousarray(W_enc_MLDH, np.float32).reshape(_D, _H)
    W_dec = np.ascontiguousarray(W_dec_HMLD, np.float32).reshape(_H, _D)

    nc = _get_nc(BC=B // N_CORES)
    in_maps = prepare_inputs(X, W_enc, W_dec, b_enc_H, b_dec_MLD)
    res = run_bass_kernel_spmd(nc, in_maps, core_ids=list(range(N_CORES)), **run_kw)
    out = np.concatenate([res.results[c]["out"] for c in range(N_CORES)], axis=0)
    if run_kw.get("trace"):
        kernel.last_result = res
    return out.reshape(B, _M, _L, _DM).astype(np.float32)
